# revision 9
# baseline (speedup 1.0000x reference)
"""Trainium2 Bass kernel for nn_MHA_42391327211690.

MHA: B=1, S=2048, E=2048, H=32 q-heads, HKV=8 kv-heads, D=64, RoPE(rot=64,
GPT-NeoX style) on q/k, causal GQA attention, out-projection with bias.

Distribution (8 NeuronCores, tensor-parallel by heads):
  - core i computes q-heads 4i..4i+3 and kv-head i (Wqkv column-sharded),
  - attention entirely local (GQA groups align with the shard),
  - AllToAll redistributes ctx^T from head-sharded to sequence-sharded,
  - out-projection computed per-core for its 256-row sequence slice
    (weights replicated), host concatenates the slices.

v4 schedule: the PE is kept streaming (HAM clock-gate warm) by software-
pipelining the attention t-loop (scores for t+2 issued before AV of t) and
filling every exp-wait gap with QKV / out-projection matmul chunks from a
fill queue.  Head-pair 0 is processed first and shipped (AllToAll #0) while
head-pair 1's attention runs; out-proj pass 1 (pair-0 features) fills the
last pair-1 attention group; pass 2 follows AllToAll #1.  QKV bias adds run
on the vector engine (broadcast bias tiles) so the scalar engine does
nothing but the softmax exp; the causal mask is accumulated on the PE
(-1e9 upper-triangle via trm^T @ I).  A2A staging and ctxF gathers are
single strided DMAs instead of per-block copies.
"""

from collections import deque
from contextlib import ExitStack

import numpy as np
import ml_dtypes

import concourse.bass as bass
import concourse.bacc as bacc
import concourse.tile as tile
from concourse import mybir
from concourse.bass_utils import run_bass_kernel_spmd

F32 = mybir.dt.float32
BF16 = mybir.dt.bfloat16
AF = mybir.ActivationFunctionType
ALU = mybir.AluOpType

B, E = 1, 2048
H, HKV, D = 32, 8, 64
ROT, BASE = 64, 10000.0
NCORES = 8
HL = H // NCORES            # 4 local q heads
OPL = (HL + 2) * D          # 384 local qkv output rows (q | k | v)
SCALE = float(D) ** -0.5
QCH = 4                     # e-tiles per qkv fill chunk
OCH = 4                     # f-tiles per out-proj fill chunk


def build_nc(S=2048, n_cores=NCORES):
    """Build the SPMD Bass program (identical on every core)."""
    SEG = S // n_cores      # per-core output sequence slice
    NT = S // 128           # key blocks
    NG = S // 512           # sq groups of 512
    NE = E // 128           # contraction tiles for qkv / out proj

    nc = bacc.Bacc("TRN2", target_bir_lowering=False, debug=False,
                   num_devices=n_cores)

    xT = nc.dram_tensor("xT", [E, S], BF16, kind="ExternalInput")
    wqkvT = nc.dram_tensor("wqkvT", [E, OPL], BF16, kind="ExternalInput")
    bbc_d = nc.dram_tensor("bbc", [OPL, 512], BF16, kind="ExternalInput")
    cc_d = nc.dram_tensor("cc", [128, S], BF16, kind="ExternalInput")
    ss_d = nc.dram_tensor("ss", [128, S], BF16, kind="ExternalInput")
    trm_d = nc.dram_tensor("trm", [128, 128], BF16, kind="ExternalInput")
    id128_d = nc.dram_tensor("id128", [128, 128], BF16, kind="ExternalInput")
    id_d = nc.dram_tensor("ident", [128, 64], F32, kind="ExternalInput")
    woT = nc.dram_tensor("woT", [E, E], BF16, kind="ExternalInput")
    outb_d = nc.dram_tensor("outb", [128, E], F32, kind="ExternalInput")
    outS = nc.dram_tensor("outS", [SEG, E], F32, kind="ExternalOutput")

    with tile.TileContext(nc) as tc, ExitStack() as ctx:
        # ------------------------- pools --------------------------------
        consts = ctx.enter_context(tc.tile_pool(name="consts", bufs=1))
        trm = consts.tile([128, 128], BF16)
        id128 = consts.tile([128, 128], BF16)
        ident = consts.tile([128, 64], F32)
        cc = consts.tile([128, S], BF16, tag="cc", name="cc_sb")
        ss = consts.tile([128, S], BF16, tag="ss", name="ss_sb")
        bb = [consts.tile([128, 512], BF16, tag=f"bb{j}", name=f"bb{j}")
              for j in range(3)]

        qkv_pool = ctx.enter_context(tc.tile_pool(name="qkv", bufs=1))
        q_sb = [qkv_pool.tile([128, S], BF16, tag=f"q{i}", name=f"qsb{i}")
                for i in range(HL // 2)]
        kdup = qkv_pool.tile([128, S], BF16, tag="kdup")
        v_sb = [qkv_pool.tile([128, 2 * D], BF16, tag=f"v{t}", name=f"vsb{t}")
                for t in range(NT)]
        ctx_sb = [qkv_pool.tile([128, S], BF16, tag=f"c{i}", name=f"ctxsb{i}")
                  for i in range(HL)]

        wo_pool = ctx.enter_context(tc.tile_pool(name="wo", bufs=1, side="right"))
        wo_sb = [wo_pool.tile([128, E], BF16, tag=f"wo{f}", name=f"wo{f}")
                 for f in range(NE)]
        ctxF = [wo_pool.tile([128, 8 * SEG], BF16, tag=f"cf{p}", name=f"cf{p}")
                for p in range(2)]

        rope_pool = ctx.enter_context(tc.tile_pool(name="rope", bufs=2))
        exps = ctx.enter_context(tc.tile_pool(name="exps", bufs=3))
        rcp = ctx.enter_context(tc.tile_pool(name="rcp", bufs=1))

        ps_s = ctx.enter_context(tc.tile_pool(name="pss", bufs=2, space="PSUM"))
        ps_c = ctx.enter_context(tc.tile_pool(name="psc", bufs=1, space="PSUM"))
        ps_w = ctx.enter_context(tc.tile_pool(name="psw", bufs=2, space="PSUM"))

        dram = ctx.enter_context(tc.tile_pool(name="dram", bufs=1, space="DRAM"))
        a2a_in = [dram.tile([n_cores * 128, SEG], BF16, tag=f"ai{p}",
                            name=f"a2ain{p}") for p in range(2)]
        a2a_out = [dram.tile([n_cores * 128, SEG], BF16, tag=f"ao{p}",
                             name=f"a2aout{p}") for p in range(2)]

        # ---------------------- input DMAs ------------------------------
        ab = ExitStack()  # x pool: closed after QKV to free space for out_sb
        x_pool = ab.enter_context(tc.tile_pool(name="x", bufs=1))
        wq_sb = [x_pool.tile([128, OPL], BF16, tag=f"wq{e}", name=f"wqsb{e}")
                 for e in range(NE)]
        xs = {}

        def _x_tile(sg, e):
            xs[(sg, e)] = x_pool.tile([128, 512], BF16, tag=f"x{e}", bufs=2,
                                      name=f"xs{sg}_{e}")
            nc.sync.dma_start(
                xs[(sg, e)][:],
                xT[e * 128:(e + 1) * 128, sg * 512:(sg + 1) * 512])

        for e in range(2):
            nc.sync.dma_start(wq_sb[e][:], wqkvT[e * 128:(e + 1) * 128, :])
            _x_tile(0, e)
        for j in range(3):
            nc.sync.dma_start(bb[j][:], bbc_d[j * 128:(j + 1) * 128, :])
        nc.sync.dma_start(ident[:], id_d[:])
        nc.sync.dma_start(trm[:], trm_d[:])
        nc.sync.dma_start(id128[:], id128_d[:])
        for e in range(2, NE):
            nc.sync.dma_start(wq_sb[e][:], wqkvT[e * 128:(e + 1) * 128, :])
            _x_tile(0, e)
        nc.sync.dma_start(cc[:], cc_d[:, :])
        nc.sync.dma_start(ss[:], ss_d[:, :])
        for sg in range(1, NG):
            for e in range(NE):
                _x_tile(sg, e)
        for f in range(NE):
            nc.sync.dma_start(wo_sb[f][:], woT[f * 128:(f + 1) * 128, :])

        # warm the ACT exp table early; ones columns of the v tiles
        warm = consts.tile([128, 1], F32, tag="warm")
        nc.scalar.activation(warm[:], bb[0][:, 0:1], AF.Exp, scale=0.0)
        for t in range(NT):
            nc.vector.memset(v_sb[t][:, 0:D], 1.0)

        # ---------------------- fill queue ------------------------------
        fillq = deque()

        def fill_pop(n=1):
            for _ in range(n):
                if not fillq:
                    return
                fillq.popleft()[1]()

        def fill_drain(tag):
            last = -1
            for i, (t_, _) in enumerate(fillq):
                if t_ == tag:
                    last = i
            for _ in range(last + 1):
                fillq.popleft()[1]()

        # ---------------------- qkv + rope ------------------------------
        def rope_q(sg, qi):
            sgs = slice(sg * 512, (sg + 1) * 512)
            qt = q_sb[qi]
            qs = rope_pool.tile([128, 512], BF16, tag="qs")
            for b in range(4):
                nc.gpsimd.dma_start(
                    qs[b * 32:(b + 1) * 32, :],
                    qt[(b ^ 1) * 32:((b ^ 1) + 1) * 32, sgs])
            t1 = rope_pool.tile([128, 512], BF16, tag="t1")
            t2 = rope_pool.tile([128, 512], BF16, tag="t2")
            nc.vector.tensor_mul(t1[:], qt[:, sgs], cc[:, sgs])
            nc.vector.tensor_mul(t2[:], qs[:], ss[:, sgs])
            nc.vector.tensor_add(qt[:, sgs], t1[:], t2[:])

        def rope_k(sg, ksrc):
            sgs = slice(sg * 512, (sg + 1) * 512)
            ks = rope_pool.tile([64, 512], BF16, tag="ks")
            for b in range(2):
                nc.gpsimd.dma_start(
                    ks[b * 32:(b + 1) * 32, :],
                    ksrc[(b ^ 1) * 32:((b ^ 1) + 1) * 32, :])
            t1 = rope_pool.tile([64, 512], BF16, tag="kt1")
            t2 = rope_pool.tile([64, 512], BF16, tag="kt2")
            nc.vector.tensor_mul(t1[:], ksrc[:], cc[0:64, sgs])
            nc.vector.tensor_mul(t2[:], ks[:], ss[0:64, sgs])
            nc.vector.tensor_add(kdup[0:64, sgs], t1[:], t2[:])
            nc.gpsimd.dma_start(kdup[64:128, sgs], kdup[0:64, sgs])

        def qkv_chunk(sg, j, e0, ps_ref):
            if e0 == 0:
                ps_ref[0] = ps_w.tile([128, 512], F32, tag="w",
                                      name=f"qps{sg}_{j}")
            ps = ps_ref[0]
            for e in range(e0, e0 + QCH):
                nc.tensor.matmul(ps[:], wq_sb[e][:, j * 128:(j + 1) * 128],
                                 xs[(sg, e)][:],
                                 start=(e == 0), stop=(e == NE - 1))
            if e0 + QCH == NE:
                qkv_epilogue(sg, j, ps)

        def qkv_epilogue(sg, j, ps):
            sgs = slice(sg * 512, (sg + 1) * 512)
            if j < 2:
                nc.vector.tensor_add(q_sb[j][:, sgs], ps[:], bb[j][:])
                rope_q(sg, j)
                return
            # j == 2: k rows 0:64, v rows 64:128
            kst = rope_pool.tile([64, 512], BF16, tag="kst")
            nc.vector.tensor_add(kst[:], ps[0:64, :], bb[2][0:64, :])
            vst = rope_pool.tile([128, 512], F32, tag="vst")
            nc.vector.tensor_add(vst[64:128, :], ps[64:128, :],
                                 bb[2][64:128, :])
            rope_k(sg, kst)
            for c in range(4):
                t = sg * 4 + c
                pvt = ps_w.tile([128, 512], F32, tag="w", name=f"pvt{t}")
                nc.tensor.transpose(pvt[:, 0:64],
                                    vst[64:128, c * 128:(c + 1) * 128],
                                    ident[64:128, :])
                nc.vector.tensor_copy(v_sb[t][:, D:2 * D], pvt[:, 0:64])

        for sg in range(NG):
            for j in (0, 2, 1):
                ps_ref = [None]
                for e0 in range(0, NE, QCH):
                    fillq.append((f"q{sg}",
                                  (lambda sg=sg, j=j, e0=e0, r=ps_ref:
                                   qkv_chunk(sg, j, e0, r))))

        # ---------------------- attention -------------------------------
        def attn_group(hp, g, budget):
            qt = q_sb[hp]
            gs = slice(g * 512, (g + 1) * 512)
            ntb = 4 * g + 4
            pc = ps_c.tile([128, 1024], F32, tag="ctx")
            exs = {}

            def S(t):
                j = t - 4 * g
                c0 = max(0, j) * 128
                ts_ = slice(t * 128, (t + 1) * 128)
                pss = ps_s.tile([128, 1024], F32, tag="s")
                nc.tensor.matmul(
                    pss[:, c0:512], kdup[0:64, ts_],
                    qt[0:64, g * 512 + c0:(g + 1) * 512],
                    start=True, stop=(j < 0))
                nc.tensor.matmul(
                    pss[:, 512 + c0:1024], kdup[64:128, ts_],
                    qt[64:128, g * 512 + c0:(g + 1) * 512],
                    start=True, stop=(j < 0))
                if j >= 0:
                    nc.tensor.matmul(pss[:, c0:c0 + 128], trm[:], id128[:],
                                     start=False, stop=True)
                    nc.tensor.matmul(pss[:, 512 + c0:512 + c0 + 128],
                                     trm[:], id128[:],
                                     start=False, stop=True)
                ex = exps.tile([128, 1024], BF16, tag="e")
                if j <= 0:
                    nc.scalar.activation(ex[:], pss[:], AF.Exp, scale=SCALE)
                else:
                    ex_r = ex[:].rearrange("p (k c) -> p k c", k=2)
                    ps_r = pss[:].rearrange("p (k c) -> p k c", k=2)
                    nc.scalar.activation(ex_r[:, :, c0:], ps_r[:, :, c0:],
                                         AF.Exp, scale=SCALE)
                    if j == 3:
                        nc.vector.memset(ex_r[:, :, :c0], 0.0)
                exs[t] = ex

            def A(t):
                j = t - 4 * g
                c0 = max(0, j) * 128
                ex = exs.pop(t)
                if j in (1, 2):
                    nc.tensor.matmul(pc[:, c0:512], v_sb[t][:],
                                     ex[:, c0:512],
                                     start=False, stop=False)
                    nc.tensor.matmul(pc[:, 512 + c0:1024], v_sb[t][:],
                                     ex[:, 512 + c0:1024],
                                     start=False, stop=False)
                else:
                    nc.tensor.matmul(pc[:, 0:512], v_sb[t][:],
                                     ex[:, 0:512],
                                     start=(t == 0), stop=(t == ntb - 1))
                    nc.tensor.matmul(pc[:, 512:1024], v_sb[t][:],
                                     ex[:, 512:1024],
                                     start=(t == 0), stop=(t == ntb - 1))

            S(0)
            if ntb > 1:
                S(1)
            for t in range(ntb):
                fill_pop(budget)
                A(t)
                if t + 2 < ntb:
                    S(t + 2)
            # denominators on pc rows 0:64; reciprocal at base 0, one DMA
            # realigns to the ctx rows 64:128
            rb = rcp.tile([64, 1024], F32, tag="rb")
            rscr = rcp.tile([64, 1024], F32, tag="rscr")
            nc.vector.reciprocal_approx_accurate(rb[:], pc[0:64, :], rscr[:])
            rbh = rcp.tile([128, 1024], F32, tag="rbh")
            nc.gpsimd.dma_start(rbh[64:128, :], rb[:])
            nc.vector.tensor_mul(ctx_sb[2 * hp][64:128, gs],
                                 pc[64:128, 0:512], rbh[64:128, 0:512])
            nc.vector.tensor_mul(ctx_sb[2 * hp + 1][64:128, gs],
                                 pc[64:128, 512:1024], rbh[64:128, 512:1024])

        def ship(hp):
            dview = a2a_in[hp][:].rearrange("(j h r) c -> h r j c",
                                            j=n_cores, h=2)
            for h2 in range(2):
                sview = ctx_sb[2 * hp + h2][64:128, :].rearrange(
                    "p (j c) -> p j c", j=n_cores)
                nc.gpsimd.dma_start(dview[h2], sview)
            with tc.high_priority():
                nc.gpsimd.collective_compute(
                    "AllToAll", ALU.bypass,
                    replica_groups=[list(range(n_cores))],
                    ins=[a2a_in[hp][:]], outs=[a2a_out[hp][:]])

        def load_ctxF(p):
            nc.gpsimd.dma_start(
                ctxF[p][:].rearrange("r (c s) -> r c s", c=8),
                a2a_out[p][:].rearrange("(c r) s -> r c s", c=8))

        # ---------------------- out projection --------------------------
        def op_chunk(p, eg, st, f0, ps_ref):
            if f0 == 0:
                ps_ref[0] = ps_w.tile([128, 512], F32, tag="w",
                                      name=f"po{p}_{eg}_{st}")
            po = ps_ref[0]
            for fi in range(f0, f0 + OCH):
                f = 2 * fi + p
                lhsT = ctxF[p][:, fi * SEG + st * 128: fi * SEG + st * 128 + 128]
                nc.tensor.matmul(po[:], lhsT,
                                 wo_sb[f][:, eg * 512:(eg + 1) * 512],
                                 start=(fi == 0), stop=(fi == 7))
            if f0 + OCH == 8:
                osl = late["out"][st][:, eg * 512:(eg + 1) * 512]
                if p == 0:
                    nc.vector.scalar_tensor_tensor(
                        osl, po[:], 1.0,
                        late["outb"][:, eg * 512:(eg + 1) * 512],
                        ALU.mult, ALU.add)
                else:
                    nc.vector.scalar_tensor_tensor(
                        osl, po[:], 1.0, osl, ALU.mult, ALU.add)
                    if eg == 3:
                        nc.sync.dma_start(outS[st * 128:(st + 1) * 128, :],
                                          late["out"][st][:])

        def push_pass(p, tag):
            for st in range(SEG // 128):
                for eg in range(4):
                    ps_ref = [None]
                    for f0 in range(0, 8, OCH):
                        fillq.append((tag,
                                      (lambda p=p, eg=eg, st=st, f0=f0,
                                       r=ps_ref: op_chunk(p, eg, st, f0, r))))

        # ---------------------- master schedule -------------------------
        late = {}
        for g in range(NG):
            fill_drain(f"q{g}")
            attn_group(0, g, budget=1)
        fill_drain("q3")
        ship(0)
        ab.close()  # release x/wq SBUF for the output tiles
        late_pool = ctx.enter_context(tc.tile_pool(name="late", bufs=1))
        late["outb"] = late_pool.tile([128, E], F32, tag="outb",
                                      name="outb_sb")
        late["out"] = [late_pool.tile([128, E], F32, tag=f"os{st}",
                                      name=f"osb{st}")
                       for st in range(SEG // 128)]
        nc.sync.dma_start(late["outb"][:], outb_d[:])
        attn_group(1, 0, budget=1)
        attn_group(1, 1, budget=1)
        attn_group(1, 2, budget=1)
        load_ctxF(0)
        push_pass(0, "p1")
        attn_group(1, 3, budget=1)
        fill_drain("p1")
        ship(1)
        load_ctxF(1)
        push_pass(1, "p2")
        fill_drain("p2")

    nc.compile()
    return nc


def shard_inputs(hidden_states, Wqkv_w, Wqkv_b, out_w, out_b, S=2048,
                 n_cores=NCORES):
    """Host-side sharding: returns per-core input maps."""
    bf16 = ml_dtypes.bfloat16
    x = np.asarray(hidden_states, np.float32).reshape(S, E)
    xT = np.ascontiguousarray(x.T).astype(bf16)
    Wqkv_w = np.asarray(Wqkv_w, np.float32)
    Wqkv_b = np.asarray(Wqkv_b, np.float32)
    woT = np.ascontiguousarray(np.asarray(out_w, np.float32).T).astype(bf16)
    outb = np.ascontiguousarray(np.broadcast_to(
        np.asarray(out_b, np.float32).reshape(1, E), (128, E)))

    inv = (1.0 / (BASE ** (np.arange(0, ROT, 2, dtype=np.float64) / ROT)))
    t = np.arange(S, dtype=np.float64)
    freqs = np.outer(t, inv)                      # [S, 32]
    cT = np.cos(freqs).T.astype(np.float32)       # [32, S]
    sT = np.sin(freqs).T.astype(np.float32)
    cc = np.tile(cT, (4, 1)).astype(bf16)         # [128, S]
    ss = np.concatenate([-sT, sT, -sT, sT], axis=0).astype(bf16)
    # trm[k, m] = -1e9 where key row m > query col k would be masked:
    # accumulated as trm^T @ I, giving scores[t, c] += trm[c, t]
    mask = (np.arange(128)[:, None] > np.arange(128)[None, :])  # t > c strict
    trm = np.ascontiguousarray((mask.T * (-1e9)).astype(np.float32)).astype(bf16)
    id128 = np.eye(128, dtype=np.float32).astype(bf16)
    ident = np.vstack([np.eye(64, dtype=np.float32)] * 2)

    in_maps = []
    for i in range(n_cores):
        hq = H // n_cores
        wq = Wqkv_w[i * hq * D:(i + 1) * hq * D]          # [256, E]
        wk = Wqkv_w[H * D + i * D: H * D + (i + 1) * D]   # [64, E]
        wv = Wqkv_w[(H + HKV) * D + i * D: (H + HKV) * D + (i + 1) * D]
        w_local = np.concatenate([wq, wk, wv], axis=0)    # [384, E]
        b_local = np.concatenate([
            Wqkv_b[i * hq * D:(i + 1) * hq * D],
            Wqkv_b[H * D + i * D: H * D + (i + 1) * D],
            Wqkv_b[(H + HKV) * D + i * D: (H + HKV) * D + (i + 1) * D]])
        bbc = np.ascontiguousarray(np.broadcast_to(
            b_local.reshape(OPL, 1), (OPL, 512))).astype(bf16)
        in_maps.append({
            "xT": xT,
            "wqkvT": np.ascontiguousarray(w_local.T).astype(bf16),
            "bbc": bbc,
            "cc": cc, "ss": ss, "trm": trm, "id128": id128, "ident": ident,
            "woT": woT, "outb": outb,
        })
    return in_maps


def assemble(results, S=2048, n_cores=NCORES):
    out = np.concatenate([r["outS"] for r in results], axis=0)
    return out.reshape(B, S, E).astype(np.float32)


_NC_CACHE = {}


def _get_nc(S=2048):
    if S not in _NC_CACHE:
        _NC_CACHE[S] = build_nc(S=S)
    return _NC_CACHE[S]


def kernel(hidden_states, Wqkv_w, Wqkv_b, out_w, out_b, _trace=False):
    in_maps = shard_inputs(hidden_states, Wqkv_w, Wqkv_b, out_w, out_b)
    nc = _get_nc()
    res = run_bass_kernel_spmd(nc, in_maps, core_ids=list(range(NCORES)),
                               trace=_trace)
    out = assemble(res.results)
    if _trace:
        kernel.last_results = res
    return out
